# revision 4
# baseline (speedup 1.0000x reference)
"""MC Soft Contrastive Loss on 8 Trainium2 NeuronCores.

Math: for each (i, j) image/caption pair the reference computes
  nll_ij = log(K^2) - logsumexp_{kl}( m_ij * s - logaddexp(s, -s) ),  s = shift - ns * dist
Using exp(m*s - logaddexp(s,-s)) = sigmoid(2*m*s), that inner term is
  log sum_{kl} sigmoid(2 * m_ij * s_ijkl).
For m = -1 (off-diagonal), sigmoid(-2s) >= sigmoid(-2*shift) > 0 so the plain
sum is always finite and safe.  Only the N diagonal pairs (m = +1) can
underflow and need a max-subtracted logsumexp, done on the gathered diagonal
distances.

Sharding: row-parallel over image samples (64 per core), every core holds all
caption samples.  Each core computes dist for its [64*K, N*K] block via one
big matmul using the Gram identity with an augmented contraction that adds
|a|^2 + |b|^2 directly in PSUM, then sigmoid + block-sum + log, and emits
  poff  [64, 1]  : per-image-row sum over j of log(sum_kl sigmoid(.)) with the
                   diagonal entry masked to 0
  gdiag [128,32] : raw diagonal-block distances for the stable diagonal path
The final scalar reduction over tiny per-core outputs happens on the host.
"""

import numpy as np
import ml_dtypes

import concourse.bass as bass
import concourse.tile as tile
from concourse import bacc, mybir
from concourse.bass_utils import run_bass_kernel_spmd

N, K, D = 512, 8, 1024
NCORES = 8
R = N // NCORES            # image rows per core
DC = D // 128              # contraction chunks
MC = R * K // 128          # m-chunks (rows of the pair grid)
NCH = N * K // 512         # n-chunks (columns of the pair grid)
IPC = 128 // K             # image samples per m-chunk (16)

f32 = mybir.dt.float32
bf16 = mybir.dt.bfloat16
BF = ml_dtypes.bfloat16

_CACHE = {}


def _build():
    nc = bacc.Bacc("TRN2", target_bir_lowering=False, debug=False,
                   num_devices=NCORES)

    ecapT = nc.dram_tensor("ecapT", [D, N * K], bf16, kind="ExternalInput")
    csigT = nc.dram_tensor("csigT", [D, N], f32, kind="ExternalInput")
    cmeanT = nc.dram_tensor("cmeanT", [D, N], bf16, kind="ExternalInput")
    eimgT = nc.dram_tensor("eimgT", [D, R * K], bf16, kind="ExternalInput")
    isigT = nc.dram_tensor("isigT", [D, R], f32, kind="ExternalInput")
    imeanT = nc.dram_tensor("imeanT", [D, R], f32, kind="ExternalInput")
    selw = nc.dram_tensor("selw", [128, MC * R], f32, kind="ExternalInput")
    negeye = nc.dram_tensor("negeye", [R, R], f32, kind="ExternalInput")
    shift = nc.dram_tensor("shift", [1], f32, kind="ExternalInput")
    nscale = nc.dram_tensor("nscale", [1], f32, kind="ExternalInput")

    poff = nc.dram_tensor("poff", [R, 1], f32, kind="ExternalOutput")
    gdist = nc.dram_tensor("gdist", [MC * 128, 512], f32, kind="ExternalOutput")

    TT = mybir.AluOpType
    AF = mybir.ActivationFunctionType

    with tile.TileContext(nc) as tc:
        with tc.tile_pool(name="big", bufs=1) as big, \
             tc.tile_pool(name="sm", bufs=1) as sm, \
             tc.tile_pool(name="wk", bufs=3) as wk, \
             tc.tile_pool(name="ps2", bufs=2, space="PSUM") as ps2, \
             tc.tile_pool(name="ps1", bufs=1, space="PSUM") as ps1:

            # ---- constants ----
            t_ns = sm.tile([128, 1], f32, tag="t_ns")
            nc.sync.dma_start(out=t_ns, in_=nscale.ap().to_broadcast((128, 1)))
            t_sh = sm.tile([128, 1], f32, tag="t_sh")
            nc.sync.dma_start(out=t_sh, in_=shift.ap().to_broadcast((128, 1)))
            ns2 = sm.tile([128, 1], f32, tag="ns2")
            nc.vector.tensor_scalar_mul(ns2, t_ns, 2.0)
            sh2 = sm.tile([128, 1], f32, tag="sh2")
            nc.vector.tensor_scalar_mul(sh2, t_sh, -2.0)
            t_sel = sm.tile([128, MC * R], f32, tag="t_sel")
            nc.sync.dma_start(out=t_sel, in_=selw[:])
            t_neye = sm.tile([R, R], f32, tag="t_neye")
            nc.sync.dma_start(out=t_neye, in_=negeye[:])
            oq = sm.tile([128, 1], bf16, tag="oq")
            nc.vector.memset(oq, 0.25)
            o1 = sm.tile([128, 1], bf16, tag="o1")
            nc.vector.memset(o1, 1.0)

            # ---- image side: aT = -2 * (mean + eps * exp(sig)), |a|^2/4 sums
            aT = []
            sa_ps = ps1.tile([1, R * K], f32, tag="sa")
            for dc in range(DC):
                t_is = wk.tile([128, R], f32, tag="t_is")
                nc.sync.dma_start(out=t_is, in_=isigT[dc * 128:(dc + 1) * 128, :])
                t_ex = wk.tile([128, R], f32, tag="t_ex")
                nc.scalar.activation(out=t_ex, in_=t_is, func=AF.Exp)
                sigX = wk.tile([128, R], bf16, tag="sigX")
                nc.vector.tensor_scalar_mul(sigX, t_ex, -2.0)
                t_im = wk.tile([128, R], f32, tag="t_im")
                nc.sync.dma_start(out=t_im, in_=imeanT[dc * 128:(dc + 1) * 128, :])
                meanX = wk.tile([128, R], bf16, tag="meanX")
                nc.vector.tensor_scalar_mul(meanX, t_im, -2.0)

                a_dc = big.tile([128, R * K], bf16, tag=f"aT{dc}")
                nc.sync.dma_start(out=a_dc, in_=eimgT[dc * 128:(dc + 1) * 128, :])
                a3 = a_dc.rearrange("p (i k) -> p i k", k=K)
                nc.vector.tensor_tensor(out=a3, in0=a3,
                                        in1=sigX.unsqueeze(2).to_broadcast((128, R, K)),
                                        op=TT.mult)
                nc.vector.tensor_tensor(out=a3, in0=a3,
                                        in1=meanX.unsqueeze(2).to_broadcast((128, R, K)),
                                        op=TT.add)
                asq = wk.tile([128, R * K], bf16, tag="asq")
                nc.vector.tensor_tensor(out=asq, in0=a_dc, in1=a_dc, op=TT.mult)
                nc.tensor.matmul(sa_ps, lhsT=oq, rhs=asq,
                                 start=(dc == 0), stop=(dc == DC - 1))
                aT.append(a_dc)

            # ---- caption side: bT = mean + eps * exp(sig) ----
            bT = []
            for dc in range(DC):
                t_cs = wk.tile([128, N], f32, tag="t_cs")
                nc.sync.dma_start(out=t_cs, in_=csigT[dc * 128:(dc + 1) * 128, :])
                cex = wk.tile([128, N], bf16, tag="cex")
                nc.scalar.activation(out=cex, in_=t_cs, func=AF.Exp)
                t_cm = wk.tile([128, N], bf16, tag="t_cm")
                nc.sync.dma_start(out=t_cm, in_=cmeanT[dc * 128:(dc + 1) * 128, :])

                b_dc = big.tile([128, N * K], bf16, tag=f"bT{dc}")
                nc.sync.dma_start(out=b_dc, in_=ecapT[dc * 128:(dc + 1) * 128, :])
                b3 = b_dc.rearrange("p (j l) -> p j l", l=K)
                nc.vector.tensor_tensor(out=b3, in0=b3,
                                        in1=cex.unsqueeze(2).to_broadcast((128, N, K)),
                                        op=TT.mult)
                nc.vector.tensor_tensor(out=b3, in0=b3,
                                        in1=t_cm.unsqueeze(2).to_broadcast((128, N, K)),
                                        op=TT.add)
                bT.append(b_dc)

            # ---- augmented contraction rows: [sa_hi, sa_lo, 1, 1] x [1, 1, sb_hi, sb_lo]
            aTaug = sm.tile([4, R * K], bf16, tag="aTaug")
            nc.vector.memset(aTaug, 1.0)
            sa_hi = sm.tile([1, R * K], bf16, tag="sa_hi")
            nc.vector.tensor_copy(out=sa_hi, in_=sa_ps)
            sa_h32 = sm.tile([1, R * K], f32, tag="sa_h32")
            nc.vector.tensor_copy(out=sa_h32, in_=sa_hi)
            sa_lo = sm.tile([1, R * K], bf16, tag="sa_lo")
            nc.vector.tensor_tensor(out=sa_lo, in0=sa_ps, in1=sa_h32, op=TT.subtract)
            nc.sync.dma_start(out=aTaug[0:1, :], in_=sa_hi)
            nc.sync.dma_start(out=aTaug[1:2, :], in_=sa_lo)

            bTaug = sm.tile([4, N * K], bf16, tag="bTaug")
            nc.vector.memset(bTaug, 1.0)
            for nch in range(NCH):
                sb_ps = ps2.tile([1, 512], f32, tag="sb")
                for dc in range(DC):
                    bsq = wk.tile([128, 512], bf16, tag="bsq")
                    nc.vector.tensor_tensor(out=bsq,
                                            in0=bT[dc][:, nch * 512:(nch + 1) * 512],
                                            in1=bT[dc][:, nch * 512:(nch + 1) * 512],
                                            op=TT.mult)
                    nc.tensor.matmul(sb_ps, lhsT=o1, rhs=bsq,
                                     start=(dc == 0), stop=(dc == DC - 1))
                sb_hi = wk.tile([1, 512], bf16, tag="sb_hi")
                nc.vector.tensor_copy(out=sb_hi, in_=sb_ps)
                sb_h32 = wk.tile([1, 512], f32, tag="sb_h32")
                nc.vector.tensor_copy(out=sb_h32, in_=sb_hi)
                sb_lo = wk.tile([1, 512], bf16, tag="sb_lo")
                nc.vector.tensor_tensor(out=sb_lo, in0=sb_ps, in1=sb_h32,
                                        op=TT.subtract)
                nc.sync.dma_start(out=bTaug[2:3, nch * 512:(nch + 1) * 512], in_=sb_hi)
                nc.sync.dma_start(out=bTaug[3:4, nch * 512:(nch + 1) * 512], in_=sb_lo)

            # ---- main pair grid ----
            t_slog = sm.tile([R, N], f32, tag="t_slog")
            for nch in range(NCH):
                s_ps = ps2.tile([R, R], f32, tag="S")
                for mc in range(MC):
                    d2 = ps2.tile([128, 512], f32, tag="d2")
                    for dc in range(DC):
                        nc.tensor.matmul(d2,
                                         lhsT=aT[dc][:, mc * 128:(mc + 1) * 128],
                                         rhs=bT[dc][:, nch * 512:(nch + 1) * 512],
                                         start=(dc == 0), stop=False)
                    nc.tensor.matmul(d2, lhsT=aTaug[:, mc * 128:(mc + 1) * 128],
                                     rhs=bTaug[:, nch * 512:(nch + 1) * 512],
                                     start=False, stop=True)
                    dist = wk.tile([128, 512], f32, tag="dist")
                    nc.vector.tensor_scalar_max(dist, d2, 0.0)
                    nc.scalar.activation(out=dist, in_=dist, func=AF.Sqrt)
                    if nch == 0:
                        nc.sync.dma_start(
                            out=gdist[mc * 128:(mc + 1) * 128, :], in_=dist)
                    sg = wk.tile([128, 512], bf16, tag="sg")
                    nc.scalar.activation(out=sg, in_=dist, func=AF.Sigmoid,
                                         bias=sh2, scale=ns2)
                    sgl = wk.tile([128, R], f32, tag="sgl")
                    nc.vector.tensor_reduce(out=sgl,
                                            in_=sg.rearrange("p (j l) -> p j l", l=K),
                                            axis=mybir.AxisListType.X, op=TT.add)
                    nc.tensor.matmul(s_ps, lhsT=t_sel[:, mc * R:(mc + 1) * R],
                                     rhs=sgl, start=(mc == 0), stop=(mc == MC - 1))
                nc.scalar.activation(out=t_slog[:, nch * R:(nch + 1) * R],
                                     in_=s_ps, func=AF.Ln)

            # mask own-diagonal entries (chunk 0 after the per-core roll)
            nc.vector.tensor_tensor(out=t_slog[:, 0:R], in0=t_slog[:, 0:R],
                                    in1=t_neye, op=TT.mult)
            t_poff = sm.tile([R, 1], f32, tag="t_poff")
            nc.vector.tensor_reduce(out=t_poff, in_=t_slog,
                                    axis=mybir.AxisListType.X, op=TT.add)
            nc.sync.dma_start(out=poff[:], in_=t_poff)

    nc.compile()
    return nc


def _prep_inputs(img_mean, img_logsigma, cap_mean, cap_logsigma,
                 eps_img, eps_cap, shift, negative_scale):
    img_mean = np.asarray(img_mean, np.float32)
    img_logsigma = np.asarray(img_logsigma, np.float32)
    cap_mean = np.asarray(cap_mean, np.float32)
    cap_logsigma = np.asarray(cap_logsigma, np.float32)
    eps_img = np.asarray(eps_img, np.float32)
    eps_cap = np.asarray(eps_cap, np.float32)
    shift = np.asarray(shift, np.float32).reshape(1)
    nscale = np.asarray(negative_scale, np.float32).reshape(1)

    # [D, N, K] views, j-major / l-minor in the flattened axis
    ecapT = np.ascontiguousarray(eps_cap.transpose(2, 0, 1)).astype(BF)
    csigT = np.ascontiguousarray(cap_logsigma.T)
    cmeanT = np.ascontiguousarray(cap_mean.T).astype(BF)

    selw = np.zeros((128, MC * R), np.float32)
    for mc in range(MC):
        selw[:, mc * R:(mc + 1) * R] = (
            np.arange(128)[:, None] // K == (np.arange(R)[None, :] - IPC * mc)
        ).astype(np.float32)
    negeye = (1.0 - np.eye(R)).astype(np.float32)

    in_maps = []
    for c in range(NCORES):
        rows = slice(c * R, (c + 1) * R)
        roll = np.roll(np.arange(N), -c * R)
        in_maps.append({
            "ecapT": np.ascontiguousarray(
                ecapT.reshape(D, N, K)[:, roll, :]).reshape(D, N * K),
            "csigT": np.ascontiguousarray(csigT[:, roll]),
            "cmeanT": np.ascontiguousarray(cmeanT[:, roll]),
            "eimgT": np.ascontiguousarray(
                eps_img[rows].transpose(2, 0, 1)).reshape(D, R * K).astype(BF),
            "isigT": np.ascontiguousarray(img_logsigma[rows].T),
            "imeanT": np.ascontiguousarray(img_mean[rows].T),
            "selw": selw,
            "negeye": negeye,
            "shift": shift,
            "nscale": nscale,
        })
    return in_maps


def _finish(results, shift, nscale):
    """Host-side reduction of the tiny per-core outputs to the scalar loss."""
    sh = float(np.asarray(shift).reshape(-1)[0])
    ns = float(np.asarray(nscale).reshape(-1)[0])
    total_off = 0.0
    total_diag = 0.0
    for c in range(NCORES):
        total_off += float(np.sum(np.asarray(results[c]["poff"], np.float64)))
        g = np.asarray(results[c]["gdist"], np.float64)        # [MC*128, 512]
        # row mc*128 + il*K + k, col i_wc*K + l with i_wc = IPC*mc + il
        g4 = g.reshape(MC, IPC, K, R, K)                       # [mc, il, k, j, l]
        dist = np.empty((R, K * K))
        for i in range(R):
            mc, il = divmod(i, IPC)
            dist[i] = g4[mc, il, :, i, :].reshape(K * K)
        s = sh - ns * dist
        z = -2.0 * s
        x = -(np.maximum(z, 0.0) + np.log1p(np.exp(-np.abs(z))))  # -softplus(z)
        m = x.max(axis=1, keepdims=True)
        lse = m[:, 0] + np.log(np.exp(x - m).sum(axis=1))
        total_diag += float(lse.sum())
    loss = 2.0 * (N * N * np.log(np.float32(K * K)) - total_off - total_diag)
    return np.float32(loss)


def kernel(img_mean, img_logsigma, cap_mean, cap_logsigma,
           eps_img, eps_cap, shift, negative_scale):
    if "nc" not in _CACHE:
        _CACHE["nc"] = _build()
    nc = _CACHE["nc"]
    in_maps = _prep_inputs(img_mean, img_logsigma, cap_mean, cap_logsigma,
                           eps_img, eps_cap, shift, negative_scale)
    res = run_bass_kernel_spmd(nc, in_maps, core_ids=list(range(NCORES)))
    return _finish(res.results, shift, negative_scale)


# revision 12
# speedup vs baseline: 1.2768x; 1.2768x over previous
"""MC Soft Contrastive Loss on 8 Trainium2 NeuronCores.

Math: for each (i, j) image/caption pair the reference computes
  nll_ij = log(K^2) - logsumexp_{kl}( m_ij * s - logaddexp(s, -s) ),  s = shift - ns * dist
Using exp(m*s - logaddexp(s,-s)) = sigmoid(2*m*s), that inner term is
  log sum_{kl} sigmoid(2 * m_ij * s_ijkl).
For m = -1 (off-diagonal), sigmoid(-2s) >= sigmoid(-2*shift) > 0 so the plain
sum is always finite and safe.  Only the N diagonal pairs (m = +1) can
underflow and need a max-subtracted logsumexp, done host-side on the dumped
diagonal-block distances.

Sharding: row-parallel over image samples (64 per core), every core holds all
caption samples.  Per-core pair grid is [R*K, N*K] with k-major rows
(m = k*R + i) and l-major columns (n = l*N + j, captions rolled so the core's
own 64 captions sit at j in [0, 64)).  dist^2 comes from one big bf16 matmul
whose contraction is augmented with [sa_hi, sa_lo, 1, 1] x [1, 1, sb_hi,
sb_lo] rows so |a|^2 + |b|^2 lands in PSUM with the -2ab term.  Epilogue:
relu (DVE) -> sqrt (ACT) -> sigmoid (ACT, bf16) -> selector matmul (sums k)
accumulated over all n-chunks (sums l) into one [R, N] PSUM tile ->
log -> mask own diagonal -> row-sum.  Outputs per core are tiny:
  poff  [R, 1]    row sums of log(sum_kl sigmoid) with diagonal masked
  gdist [R*K, N]  diagonal-candidate distances (own-caption columns)
The final scalar reduction happens on the host in float64.
"""

import numpy as np
import ml_dtypes

import concourse.bass as bass
import concourse.tile as tile
from concourse import bacc, mybir
from concourse.bass_utils import run_bass_kernel_spmd

N, K, D = 512, 8, 1024
NCORES = 8
R = N // NCORES            # image rows per core (64)
DC = D // 128              # contraction chunks (8)
MC = R * K // 128          # m-chunks (4)
NCH = N * K // 512         # n-chunks (8) == the K values of l
QUAD = 4                   # n-chunks per ACT batching group

f32 = mybir.dt.float32
bf16 = mybir.dt.bfloat16
BF = ml_dtypes.bfloat16

_CACHE = {}


def _build():
    nc = bacc.Bacc("TRN2", target_bir_lowering=False, debug=False,
                   num_devices=NCORES)

    ecapT = nc.dram_tensor("ecapT", [D, N * K], bf16, kind="ExternalInput")
    csigT = nc.dram_tensor("csigT", [D, N], f32, kind="ExternalInput")
    cmeanT = nc.dram_tensor("cmeanT", [D, N], bf16, kind="ExternalInput")
    eimgT = nc.dram_tensor("eimgT", [D, R * K], bf16, kind="ExternalInput")
    isigT = nc.dram_tensor("isigT", [D, R], f32, kind="ExternalInput")
    imeanT = nc.dram_tensor("imeanT", [D, R], f32, kind="ExternalInput")
    selw = nc.dram_tensor("selw", [128, R], bf16, kind="ExternalInput")
    negeye = nc.dram_tensor("negeye", [R, R], f32, kind="ExternalInput")
    shift = nc.dram_tensor("shift", [1], f32, kind="ExternalInput")
    nscale = nc.dram_tensor("nscale", [1], f32, kind="ExternalInput")

    poff = nc.dram_tensor("poff", [R, 1], f32, kind="ExternalOutput")
    gdist = nc.dram_tensor("gdist", [MC * 128, NCH * R], f32,
                           kind="ExternalOutput")

    TT = mybir.AluOpType
    AF = mybir.ActivationFunctionType

    with tile.TileContext(nc) as tc:
        with tc.tile_pool(name="big", bufs=1) as big, \
             tc.tile_pool(name="sm", bufs=1) as sm, \
             tc.tile_pool(name="wk", bufs=3) as wk, \
             tc.tile_pool(name="dl", bufs=18) as dl, \
             tc.tile_pool(name="sgp", bufs=6) as sgp, \
             tc.tile_pool(name="ps2", bufs=2, space="PSUM") as ps2, \
             tc.tile_pool(name="psd", bufs=4, space="PSUM") as psd, \
             tc.tile_pool(name="ps1", bufs=1, space="PSUM") as ps1:

            # ---- constants ----
            t_ns = sm.tile([128, 1], f32, tag="t_ns")
            nc.sync.dma_start(out=t_ns, in_=nscale.ap().to_broadcast((128, 1)))
            t_sh = sm.tile([128, 1], f32, tag="t_sh")
            nc.sync.dma_start(out=t_sh, in_=shift.ap().to_broadcast((128, 1)))
            ns2 = sm.tile([128, 1], f32, tag="ns2")
            nc.vector.tensor_scalar_mul(ns2, t_ns, 2.0)
            sh2 = sm.tile([128, 1], f32, tag="sh2")
            nc.vector.tensor_scalar_mul(sh2, t_sh, -2.0)
            t_sel = sm.tile([128, R], bf16, tag="t_sel")
            nc.sync.dma_start(out=t_sel, in_=selw[:])
            t_neye = sm.tile([R, R], f32, tag="t_neye")
            nc.sync.dma_start(out=t_neye, in_=negeye[:])
            oq = sm.tile([128, 1], bf16, tag="oq")
            nc.vector.memset(oq, 0.25)
            o1 = sm.tile([128, 1], bf16, tag="o1")
            nc.vector.memset(o1, 1.0)

            # ---- caption / image sample construction.  Caption chunk 0 is
            # emitted first so the first main matmuls can start while the
            # rest of the inputs stream in. ----
            aT = []
            bT = [None] * DC
            sa_ps = ps1.tile([1, R * K], f32, tag="sa")

            def build_b(dc):
                t_cs = wk.tile([128, N], f32, tag="t_cs")
                nc.sync.dma_start(out=t_cs, in_=csigT[dc * 128:(dc + 1) * 128, :])
                cex = wk.tile([128, N], bf16, tag="cex")
                nc.scalar.activation(out=cex, in_=t_cs, func=AF.Exp)
                t_cm = wk.tile([128, N], bf16, tag="t_cm")
                nc.sync.dma_start(out=t_cm, in_=cmeanT[dc * 128:(dc + 1) * 128, :])

                b_dc = big.tile([128, N * K], bf16, tag=f"bT{dc}")
                nparts = 2 if dc == 0 else 1
                part = N * K // nparts
                for h in range(nparts):
                    sl = slice(h * part, (h + 1) * part)
                    nc.sync.dma_start(out=b_dc[:, sl],
                                      in_=ecapT[dc * 128:(dc + 1) * 128, sl])
                    b3 = b_dc[:, sl].rearrange("p (l j) -> p l j", l=K // nparts)
                    cexb = cex.unsqueeze(1).to_broadcast((128, K // nparts, N))
                    cmb = t_cm.unsqueeze(1).to_broadcast((128, K // nparts, N))
                    nc.vector.tensor_tensor(out=b3, in0=b3, in1=cexb, op=TT.mult)
                    nc.vector.tensor_tensor(out=b3, in0=b3, in1=cmb, op=TT.add)
                bT[dc] = b_dc


            build_b(0)
            for dc in range(DC):
                t_is = wk.tile([128, R], f32, tag="t_is")
                nc.sync.dma_start(out=t_is, in_=isigT[dc * 128:(dc + 1) * 128, :])
                t_ex = wk.tile([128, R], f32, tag="t_ex")
                nc.scalar.activation(out=t_ex, in_=t_is, func=AF.Exp)
                sigX = wk.tile([128, R], bf16, tag="sigX")
                nc.vector.tensor_scalar_mul(sigX, t_ex, -2.0)
                t_im = wk.tile([128, R], f32, tag="t_im")
                nc.sync.dma_start(out=t_im, in_=imeanT[dc * 128:(dc + 1) * 128, :])
                meanX = wk.tile([128, R], bf16, tag="meanX")
                nc.vector.tensor_scalar_mul(meanX, t_im, -2.0)

                a_dc = big.tile([128, R * K], bf16, tag=f"aT{dc}")
                nc.sync.dma_start(out=a_dc, in_=eimgT[dc * 128:(dc + 1) * 128, :])
                a3 = a_dc.rearrange("p (k i) -> p k i", k=K)
                nc.vector.tensor_tensor(out=a3, in0=a3,
                                        in1=sigX.unsqueeze(1).to_broadcast((128, K, R)),
                                        op=TT.mult)
                nc.vector.tensor_tensor(out=a3, in0=a3,
                                        in1=meanX.unsqueeze(1).to_broadcast((128, K, R)),
                                        op=TT.add)
                asq = wk.tile([128, R * K], bf16, tag="asq")
                nc.vector.tensor_tensor(out=asq, in0=a_dc, in1=a_dc, op=TT.mult)
                nc.tensor.matmul(sa_ps, lhsT=oq, rhs=asq,
                                 start=(dc == 0), stop=(dc == DC - 1))
                aT.append(a_dc)

            for dc in range(1, DC):
                build_b(dc)

            # ---- augmented rows: [sa_hi, sa_lo, 1, 1] x [1, 1, sb_hi, sb_lo]
            aTaug = sm.tile([4, R * K], bf16, tag="aTaug")
            nc.vector.memset(aTaug, 1.0)
            sa_hi = sm.tile([1, R * K], bf16, tag="sa_hi")
            nc.vector.tensor_copy(out=sa_hi, in_=sa_ps)
            sa_h32 = sm.tile([1, R * K], f32, tag="sa_h32")
            nc.vector.tensor_copy(out=sa_h32, in_=sa_hi)
            sa_lo = sm.tile([1, R * K], bf16, tag="sa_lo")
            nc.vector.tensor_tensor(out=sa_lo, in0=sa_ps, in1=sa_h32, op=TT.subtract)
            nc.sync.dma_start(out=aTaug[0:1, :], in_=sa_hi)
            nc.sync.dma_start(out=aTaug[1:2, :], in_=sa_lo)

            bTaug = sm.tile([4, N * K], bf16, tag="bTaug")
            nc.vector.memset(bTaug, 1.0)
            sbrow = sm.tile([1, N * K], f32, tag="sbrow")
            for nch in range(NCH):
                sb_ps = ps2.tile([1, 512], f32, tag="sb")
                for dc in range(DC):
                    bsq = wk.tile([128, 512], bf16, tag="bsq")
                    nc.vector.tensor_tensor(out=bsq,
                                            in0=bT[dc][:, nch * 512:(nch + 1) * 512],
                                            in1=bT[dc][:, nch * 512:(nch + 1) * 512],
                                            op=TT.mult)
                    nc.tensor.matmul(sb_ps, lhsT=o1, rhs=bsq,
                                     start=(dc == 0), stop=(dc == DC - 1))
                nc.vector.tensor_copy(out=sbrow[:, nch * 512:(nch + 1) * 512],
                                      in_=sb_ps)
            sb_hi = sm.tile([1, N * K], bf16, tag="sb_hi")
            nc.vector.tensor_copy(out=sb_hi, in_=sbrow)
            sb_h32 = sm.tile([1, N * K], f32, tag="sb_h32")
            nc.vector.tensor_copy(out=sb_h32, in_=sb_hi)
            sb_lo = sm.tile([1, N * K], bf16, tag="sb_lo")
            nc.vector.tensor_tensor(out=sb_lo, in0=sbrow, in1=sb_h32,
                                    op=TT.subtract)
            nc.sync.dma_start(out=bTaug[2:3, :], in_=sb_hi)
            nc.sync.dma_start(out=bTaug[3:4, :], in_=sb_lo)

            # ---- main pair grid; S accumulates sum over k (selector) and l
            # (PSUM accumulation across all 32 (nch, mc) sigmoid tiles).
            # ACT work is loosely phase-batched per group of 8 tiles: the
            # sigmoid bias tile reads a column of the group's last dist tile
            # so the scalar engine finishes the group's sqrts before starting
            # its sigmoids (2 LUT-set loads per group instead of ~2 per tile).
            s_ps = ps1.tile([R, N], f32, tag="S")
            GROUPS = 4
            GN = NCH // GROUPS
            n_sel = 0
            prev_last_sg = None
            for grp in range(GROUPS):
                dists = []
                for nq in range(GN):
                    nch = grp * GN + nq
                    for mc in range(MC):
                        d2 = psd.tile([128, 512], f32, tag="d2")
                        for dc in range(DC):
                            nc.tensor.matmul(d2,
                                             lhsT=aT[dc][:, mc * 128:(mc + 1) * 128],
                                             rhs=bT[dc][:, nch * 512:(nch + 1) * 512],
                                             start=(dc == 0), stop=False)
                        nc.tensor.matmul(d2, lhsT=aTaug[:, mc * 128:(mc + 1) * 128],
                                         rhs=bTaug[:, nch * 512:(nch + 1) * 512],
                                         start=False, stop=True)
                        dist = dl.tile([128, 512], f32, tag="dist")
                        nc.vector.tensor_scalar_max(dist, d2, 0.0)
                        dists.append((nch, mc, dist))
                if prev_last_sg is None:
                    bias_q = 0.0
                else:
                    bias_q = sm.tile([128, 1], f32, tag=f"bq{grp}")
                    nc.vector.scalar_tensor_tensor(out=bias_q,
                                                   in0=prev_last_sg[:, 0:1],
                                                   scalar=0.0, in1=sh2,
                                                   op0=TT.mult, op1=TT.mult)
                for nch, mc, dist in dists:
                    nc.scalar.activation(out=dist, in_=dist, func=AF.Sqrt,
                                         bias=bias_q)
                    nc.gpsimd.dma_start(
                        out=gdist[mc * 128:(mc + 1) * 128, nch * R:(nch + 1) * R],
                        in_=dist[:, 0:R])
                shg = sm.tile([128, 1], f32, tag=f"shg{grp}")
                nc.vector.scalar_tensor_tensor(out=shg, in0=dists[-1][2][:, 0:1],
                                               scalar=0.0, in1=sh2,
                                               op0=TT.mult, op1=TT.add)
                sgs = []
                for nch, mc, dist in dists:
                    sg = sgp.tile([128, 512], bf16, tag="sg")
                    nc.scalar.activation(out=sg, in_=dist, func=AF.Sigmoid,
                                         bias=shg, scale=ns2)
                    sgs.append(sg)
                prev_last_sg = sgs[-1]
                for sg in sgs:
                    nc.tensor.matmul(s_ps, lhsT=t_sel, rhs=sg,
                                     start=(n_sel == 0),
                                     stop=(n_sel == NCH * MC - 1),
                                     skip_group_check=True)
                    n_sel += 1

            slog = sm.tile([R, N], f32, tag="slog")
            nc.scalar.activation(out=slog, in_=s_ps, func=AF.Ln)
            nc.vector.tensor_tensor(out=slog[:, 0:R], in0=slog[:, 0:R],
                                    in1=t_neye, op=TT.mult)
            t_poff = sm.tile([R, 1], f32, tag="t_poff")
            nc.vector.tensor_reduce(out=t_poff, in_=slog,
                                    axis=mybir.AxisListType.X, op=TT.add)
            nc.sync.dma_start(out=poff[:], in_=t_poff)

    nc.compile()
    return nc


def _prep_inputs(img_mean, img_logsigma, cap_mean, cap_logsigma,
                 eps_img, eps_cap, shift, negative_scale):
    img_mean = np.asarray(img_mean, np.float32)
    img_logsigma = np.asarray(img_logsigma, np.float32)
    cap_mean = np.asarray(cap_mean, np.float32)
    cap_logsigma = np.asarray(cap_logsigma, np.float32)
    eps_img = np.asarray(eps_img, np.float32)
    eps_cap = np.asarray(eps_cap, np.float32)
    shift = np.asarray(shift, np.float32).reshape(1)
    nscale = np.asarray(negative_scale, np.float32).reshape(1)

    # [D, K, N] l-major caption layout
    ecapT = np.ascontiguousarray(eps_cap.transpose(2, 1, 0)).astype(BF)
    csigT = np.ascontiguousarray(cap_logsigma.T)
    cmeanT = np.ascontiguousarray(cap_mean.T).astype(BF)

    selw = (np.arange(128)[:, None] % R == np.arange(R)[None, :]).astype(BF)
    negeye = (1.0 - np.eye(R)).astype(np.float32)

    in_maps = []
    for c in range(NCORES):
        rows = slice(c * R, (c + 1) * R)
        roll = np.roll(np.arange(N), -c * R)
        in_maps.append({
            "ecapT": np.ascontiguousarray(
                ecapT.reshape(D, K, N)[:, :, roll]).reshape(D, N * K),
            "csigT": np.ascontiguousarray(csigT[:, roll]),
            "cmeanT": np.ascontiguousarray(cmeanT[:, roll]),
            "eimgT": np.ascontiguousarray(
                eps_img[rows].transpose(2, 1, 0)).reshape(D, R * K).astype(BF),
            "isigT": np.ascontiguousarray(img_logsigma[rows].T),
            "imeanT": np.ascontiguousarray(img_mean[rows].T),
            "selw": selw,
            "negeye": negeye,
            "shift": shift,
            "nscale": nscale,
        })
    return in_maps


def _finish(results, shift, nscale):
    """Host-side reduction of the tiny per-core outputs to the scalar loss."""
    sh = float(np.asarray(shift).reshape(-1)[0])
    ns = float(np.asarray(nscale).reshape(-1)[0])
    total_off = 0.0
    total_diag = 0.0
    idx_i = np.arange(R)
    for c in range(NCORES):
        total_off += float(np.sum(np.asarray(results[c]["poff"], np.float64)))
        g = np.asarray(results[c]["gdist"], np.float64)   # [MC*128, NCH*R]
        # row (k//2)*128 + (k%2)*64 + i, col l*R + i  ->  dist[i, k, l]
        g5 = g.reshape(MC, 2, R, NCH, R)                  # [mc, khalf, i, l, j]
        dist = g5[:, :, idx_i, :, idx_i]                  # [i, mc, khalf, l]
        dist = dist.reshape(R, K * K)
        s = sh - ns * dist
        z = -2.0 * s
        x = -(np.maximum(z, 0.0) + np.log1p(np.exp(-np.abs(z))))  # -softplus(z)
        m = x.max(axis=1, keepdims=True)
        lse = m[:, 0] + np.log(np.exp(x - m).sum(axis=1))
        total_diag += float(lse.sum())
    loss = 2.0 * (N * N * np.log(np.float32(K * K)) - total_off - total_diag)
    return np.float32(loss)


def kernel(img_mean, img_logsigma, cap_mean, cap_logsigma,
           eps_img, eps_cap, shift, negative_scale):
    if "nc" not in _CACHE:
        _CACHE["nc"] = _build()
    nc = _CACHE["nc"]
    in_maps = _prep_inputs(img_mean, img_logsigma, cap_mean, cap_logsigma,
                           eps_img, eps_cap, shift, negative_scale)
    res = run_bass_kernel_spmd(nc, in_maps, core_ids=list(range(NCORES)))
    return _finish(res.results, shift, negative_scale)
